# revision 1
# baseline (speedup 1.0000x reference)
"""Trainium2 Bass kernel: LayerNorm -> QKV -> linear (elu+1) attention -> proj.

Data-parallel over batch: 8 batch elements, one per NeuronCore. All matmuls
in bf16 (fp32 accumulation in PSUM); LayerNorm statistics in fp32; the
projection bias is applied in fp32.

Self-contained: hardcodes shapes from the problem spec.
"""

import numpy as np
import ml_dtypes

from concourse import bass, bacc, tile, mybir
from concourse.bass import ts, ds
from concourse.bass_utils import run_bass_kernel_spmd

F32 = mybir.dt.float32
F32R = mybir.dt.float32r
BF16 = mybir.dt.bfloat16
AF = mybir.ActivationFunctionType
ALU = mybir.AluOpType

# Problem shapes
N = 4096          # tokens per batch element
D = 768           # model dim
H = 12            # heads
HD = 64           # head dim
E3 = 3 * D        # qkv width
P = 128
KT = D // P       # 6 d-tiles
NT = N // P       # 32 token tiles
CH = 8            # token chunks of 512
TPC = NT // CH    # 4 token tiles per chunk
CW = N // CH      # 512 chunk width
LN_EPS = 1e-5
EPS = 1e-6

N_CORES = 8
LDW_SKIP = True


def _build(with_qkv_bias: bool, dbg: bool = False):
    """Build the single-core program (SPMD: same NEFF on all 8 cores)."""
    nc = bacc.Bacc("TRN2", target_bir_lowering=False, debug=False,
                   num_devices=N_CORES)

    x_d = nc.dram_tensor("x", [N, D], BF16, kind="ExternalInput").ap()
    wqkvT_d = nc.dram_tensor("wqkvT", [D, E3], BF16, kind="ExternalInput").ap()
    wprojT_d = nc.dram_tensor("wprojT", [D, D], BF16, kind="ExternalInput").ap()
    emat_d = nc.dram_tensor("emat", [H, D], BF16, kind="ExternalInput").ap()
    bpack_d = nc.dram_tensor("bpack", [1, P + D], F32, kind="ExternalInput").ap()
    if with_qkv_bias:
        cqkv_d = nc.dram_tensor("cqkv", [1, E3], F32, kind="ExternalInput").ap()
    out_d = nc.dram_tensor("out", [N, D], F32, kind="ExternalOutput").ap()

    from contextlib import ExitStack
    with tile.TileContext(nc) as tc, ExitStack() as stk:
        _kernel(tc, stk, nc, x_d, wqkvT_d, wprojT_d, bpack_d, emat_d,
                cqkv_d if with_qkv_bias else None, out_d, dbg)

    nc.compile()
    return nc


def _kernel(tc, stk, nc, x_d, wqkvT_d, wprojT_d, bpack_d, emat_d, cqkv_d, out_d,
            dbg=False):
    def dump(name, tl, shape, dtype):
        if not dbg:
            return
        d = nc.dram_tensor("dbg_" + name, shape, dtype, kind="ExternalOutput").ap()
        nc.sync.dma_start(d, tl)

    from contextlib import ExitStack
    consts = stk.enter_context(tc.tile_pool(name="consts", bufs=1))
    stk1 = stk.enter_context(ExitStack())
    ppersist = stk1.enter_context(tc.tile_pool(name="ppersist", bufs=1, space="PSUM"))

    # Prefetch the first chunks' x tiles before the big weight DMAs so the
    # LN chain starts immediately (DMA queues drain in priority order).
    x_prefetch = {}
    xTp = stk.enter_context(tc.tile_pool(name="xT", bufs=3 * KT))
    xpool_early = stk1.enter_context(tc.tile_pool(name="x", bufs=10))
    for t in range(2 * TPC):
        xt = xpool_early.tile([P, D], BF16)
        nc.sync.dma_start(xt[:], x_d[ts(t, P), :])
        x_prefetch[t] = xt

    # --- resident weights ---
    wqkvT = consts.tile([P, KT, E3], BF16)
    wq_r = wqkvT_d.rearrange("(kt p) e -> p kt e", p=P)
    for kt in range(KT):
        nc.sync.dma_start(wqkvT[:, kt], wq_r[:, kt])
    wprojT = consts.tile([P, KT, D], BF16)
    wp_r = wprojT_d.rearrange("(kt p) e -> p kt e", p=P)
    for kt in range(KT):
        nc.sync.dma_start(wprojT[:, kt], wp_r[:, kt])

    # --- broadcast b_proj to [128, D] fp32 once via K=1 fp32 matmuls ---
    bproj_row = consts.tile([1, D], F32)
    nc.sync.dma_start(bproj_row[:], bpack_d[:, P:P + D])
    ones_row = consts.tile([1, P], F32)
    nc.vector.memset(ones_row[:], 1.0)
    with tc.tile_pool(name="pbias", bufs=1, space="PSUM") as pbias:
        bias_sb = consts.tile([P, D], F32)
        for j, w_ in ((0, 512), (1, 256)):
            bias_ps = pbias.tile([P, 512], F32)
            nc.tensor.matmul(bias_ps[:, :w_], ones_row[:],
                             bproj_row[:, ds(j * 512, w_)],
                             start=True, stop=True)
            nc.vector.tensor_copy(bias_sb[:, ds(j * 512, w_)], bias_ps[:, :w_])

    # --- E matrix: E[h, d] = 1 iff d in head h (for z replication) ---
    E_sb = consts.tile([H, D], BF16)
    nc.sync.dma_start(E_sb[:], emat_d)

    # zero-row for psum-bank init matmuls
    zrow = consts.tile([1, 512], BF16)
    nc.vector.memset(zrow[:], 0.0)
    ones_bf = consts.tile([1, P], BF16)
    nc.vector.memset(ones_bf[:], 1.0)

    # --- kv accumulator ---
    # pair p = h//2 -> cols [65p, 65p+65), head parity s=h%2 -> partitions
    # [64s, 64s+64). col 64 of each head block = k_sum.
    kv_ps = ppersist.tile([P, 6 * 65], F32)
    # Init the whole kv bank with one start=True matmul writing zeros: sets
    # every has_written bit so the 12 interleaved accumulation chains below
    # can all run with start=False. (start=True clears the *bank's* bits, so
    # per-chain start flags would clobber each other.)
    nc.tensor.matmul(kv_ps[:], ones_bf[:], zrow[:, 0:6 * 65], start=True,
                     stop=False, skip_group_check=True)

    xpool = xpool_early
    stat = stk1.enter_context(tc.tile_pool(name="stat", bufs=12))
    xhatp = stk1.enter_context(tc.tile_pool(name="xhat", bufs=3))
    kvps = stk1.enter_context(tc.tile_pool(name="kvps", bufs=2, space="PSUM"))
    qpsp = stk1.enter_context(tc.tile_pool(name="qpsp", bufs=1, space="PSUM"))
    evac = stk1.enter_context(tc.tile_pool(name="evac", bufs=3))

    qT_all = consts.tile([P, KT, N], BF16)
    dramp = stk1.enter_context(tc.tile_pool(name="dram", bufs=3, space="DRAM"))

    # ============ PHASE 1: LN, transpose, k/v, kv accumulation ============
    for c in range(CH):
        xts, mvs = [], []
        for tt in range(TPC):
            t = c * TPC + tt
            if t in x_prefetch:
                xt = x_prefetch.pop(t)
            else:
                xt = xpool.tile([P, D], BF16)
                nc.sync.dma_start(xt[:], x_d[ts(t, P), :])
            xts.append(xt)
            # LayerNorm stats (fp32)
            st6 = stat.tile([P, 2, 6], F32)
            nc.vector.bn_stats(st6[:, 0], xt[:, 0:D // 2])
            nc.vector.bn_stats(st6[:, 1], xt[:, D // 2:D])
            mv = stat.tile([P, 2], F32)
            nc.vector.bn_aggr(mv[:], st6[:])
            mvs.append(mv)
        # batched rstd = rsqrt(var+eps) via bit-trick seed + 3 Newton steps
        I32 = mybir.dt.int32
        veps = stat.tile([P, TPC], F32)
        for tt in range(TPC):
            nc.vector.tensor_scalar_add(veps[:, tt:tt + 1], mvs[tt][:, 1:2],
                                        LN_EPS)
        t1 = stat.tile([P, TPC], I32, tag="rs_t1")
        nc.vector.tensor_scalar(t1[:], veps[:].bitcast(I32), 1, None,
                                op0=ALU.arith_shift_right)
        rstd = stat.tile([P, TPC], F32)
        nc.vector.tensor_scalar(rstd[:].bitcast(I32), t1[:], -1, 0x5F3759DF,
                                op0=ALU.mult, op1=ALU.add)
        for _ in range(3):
            a = stat.tile([P, TPC], F32, tag="rs_a")
            nc.vector.tensor_tensor(a[:], rstd[:], rstd[:], ALU.mult)
            nc.vector.tensor_tensor(a[:], a[:], veps[:], ALU.mult)
            nc.vector.tensor_scalar(a[:], a[:], -0.5, 1.5, op0=ALU.mult,
                                    op1=ALU.add)
            nc.vector.tensor_tensor(rstd[:], rstd[:], a[:], ALU.mult)
        xhat = xhatp.tile([P, TPC, D], BF16)
        xh_dram = dramp.tile([CW, D], BF16)
        for tt in range(TPC):
            # xhat = (x - mean) * rstd   -> bf16
            nc.vector.tensor_scalar(xhat[:, tt], xts[tt][:], mvs[tt][:, 0:1],
                                    rstd[:, tt:tt + 1],
                                    op0=ALU.subtract, op1=ALU.mult)
        # single bounce DMA per chunk (one writer for the transpose reads)
        nc.sync.dma_start(xh_dram[:].rearrange("(tt p) d -> p tt d", p=P),
                          xhat[:])

        # transpose the chunk: [t, d] -> [d, t] via DRAM->SBUF DMA transpose
        # (per-kt tiles for fine-grained deps; spread across both HWDGE
        # engines so the ~1.3us descriptor writes run in parallel)
        xT = [xTp.tile([P, CW], BF16, tag="xTkt", name=f"xT_{c}_{kt}")
              for kt in range(KT)]
        for kt in range(KT):
            nc.sync.dma_start_transpose(out=xT[kt][:], in_=xh_dram[:, ts(kt, P)])
        if c == 0:
            for kt in range(KT):
                dump(f"xT0_{kt}", xT[kt][:], [P, CW], BF16)
            dump("xhd0", xh_dram[:], [CW, D], BF16)

        # --- q, directly transposed: qT[dq, t] (weights stationary) ---
        for m in range(KT):
            q_ps = qpsp.tile([P, 512], F32, tag="qps1")
            for kt in range(KT):
                nc.tensor.matmul(q_ps[:], wqkvT[:, kt, ts(m, P)], xT[kt][:],
                                 start=(kt == 0), stop=(kt == KT - 1))
            et = evac.tile([P, CW], BF16, tag="elu_e")
            nc.scalar.activation(et[:], q_ps[:], AF.Exp)
            rt = evac.tile([P, CW], BF16, tag="elu_r")
            nc.vector.tensor_scalar_max(rt[:], q_ps[:], 0.0)
            nc.vector.scalar_tensor_tensor(qT_all[:, m, ts(c, CW)], et[:], 1.0,
                                           rt[:], op0=ALU.min, op1=ALU.add)

        # --- k, v in [t, e] layout (activations stationary) ---
        for tt in range(TPC):
            t = c * TPC + tt
            kv3 = kvps.tile([P, 3 * 512], F32, tag="ph1ps")  # qkv cols [768, 2304)
            for kt in range(KT):
                for j in range(3):
                    mm = nc.tensor.matmul(
                        kv3[:, ts(j, 512)],
                        xT[kt][:, ts(tt, P)],
                        wqkvT[:, kt, ds(D + j * 512, 512)],
                        start=(kt == 0), stop=(kt == KT - 1))
                    if j > 0 and LDW_SKIP:
                        mm.ldweights = False  # same stationary as j-1
            # k = elu1(cols 0:768) = min(exp, 1) + relu   (exp/relu on ACT)
            ek = evac.tile([P, D], BF16, tag="elu_ek")
            nc.scalar.activation(ek[:], kv3[:, 0:D], AF.Exp)
            rk = evac.tile([P, D], BF16, tag="elu_rk")
            nc.vector.tensor_scalar_max(rk[:], kv3[:, 0:D], 0.0)
            ktile = evac.tile([P, D], BF16, tag="ktile")
            nc.vector.scalar_tensor_tensor(ktile[:], ek[:], 1.0, rk[:],
                                           op0=ALU.min, op1=ALU.add)
            # v' = [v_h | 1] per head: [128, 12, 65]
            vtile = evac.tile([P, H, HD + 1], BF16, tag="vtile")
            nc.vector.memset(vtile[:, :, HD:HD + 1], 1.0)
            nc.scalar.activation(
                vtile[:, :, 0:HD],
                kv3[:, D:2 * D].rearrange("p (h e) -> p h e", h=H),
                AF.Copy)
            if c == 0 and tt == 0:
                dump("ktile0", ktile[:], [P, D], BF16)
                dump("vtile0", vtile[:], [P, H, HD + 1], BF16)
            # kv accumulation: 12 heads, 2 packed per psum column block
            for h in range(H):
                p_, s_ = h // 2, h % 2
                nc.tensor.matmul(
                    kv_ps[ds(64 * s_, 64), ds(65 * p_, 65)],
                    ktile[:, ds(HD * h, HD)],
                    vtile[:, h],
                    start=False, stop=(t == NT - 1),
                    skip_group_check=True,
                    tile_position=(0, 64 * s_))

    # ================= PHASE 1.5: kv -> sbuf, Ksel ========================
    kv_sb = consts.tile([P, 6 * 65], BF16)
    nc.vector.tensor_copy(kv_sb[:], kv_ps[:])
    dump("kv", kv_sb[:], [P, 6 * 65], BF16)
    dump("qTd", qT_all[:], [P, KT, N], BF16)
    ksel = consts.tile([P, KT, H], BF16)
    nc.vector.memset(ksel[:], 0.0)
    for kt in range(KT):
        for s_ in range(2):
            h = 2 * kt + s_
            nc.vector.tensor_copy(
                ksel[ds(64 * s_, 64), kt, h:h + 1],
                kv_sb[ds(64 * s_, 64), ds(65 * kt + 64, 1)])

    stk1.close()

    zps = stk.enter_context(tc.tile_pool(name="zps", bufs=1, space="PSUM"))
    zrp = stk.enter_context(tc.tile_pool(name="zrp", bufs=1, space="PSUM"))
    atps = stk.enter_context(tc.tile_pool(name="atps", bufs=2, space="PSUM"))
    ops_ = stk.enter_context(tc.tile_pool(name="ops", bufs=2, space="PSUM"))
    ph2 = stk.enter_context(tc.tile_pool(name="ph2", bufs=3))

    # ============ PHASE 2: z, attn out, proj ==============================
    for c in range(CH):
        qT = qT_all[:, :, ts(c, CW)]
        # z_pre[h, t] = sum_d ksel[d, h] * qT[d, t]
        z_ps = zps.tile([H, CW], F32)
        for kt in range(KT):
            nc.tensor.matmul(z_ps[:], ksel[:, kt], qT[:, kt],
                             start=(kt == 0), stop=(kt == KT - 1))
        # z = 1/(z_pre + EPS) on the scalar engine (LUT reciprocal: plenty
        # accurate here -- z only scales the attention output)
        zb = ph2.tile([H, CW], BF16, tag="zb")
        nc.scalar.add_instruction(mybir.InstActivation(
            name=nc.get_next_instruction_name(),
            func=AF.Reciprocal,
            ins=[nc.scalar.lower_ap(z_ps[:]),
                 mybir.ImmediateValue(dtype=F32, value=EPS),
                 mybir.ImmediateValue(dtype=F32, value=1.0),
                 mybir.ImmediateValue(dtype=F32, value=0.0)],
            outs=[nc.scalar.lower_ap(zb[:])]))
        if c == 0:
            dump("zf0", zb[:], [H, CW], BF16)

        # qz = qT * z[head(d)]  (z replicated over head dims via E matmul)
        qz = ph2.tile([P, KT, CW], BF16, tag="qz")
        for kt in range(KT):
            zr_ps = zrp.tile([P, CW], F32)
            nc.tensor.matmul(zr_ps[:], E_sb[:, ts(kt, P)], zb[:],
                             start=True, stop=True)
            nc.vector.tensor_mul(qz[:, kt], qT[:, kt], zr_ps[:])

        # attn_T[e, t] per head pair; head parity s in its own quadrant
        attnT = ph2.tile([P, KT, CW], BF16, tag="attnT")
        for p_ in range(KT):
            at_ps = atps.tile([P, CW], F32)
            for s_ in range(2):
                nc.tensor.matmul(
                    at_ps[ds(64 * s_, 64), :],
                    kv_sb[ds(64 * s_, 64), ds(65 * p_, 64)],
                    qz[ds(64 * s_, 64), p_],
                    start=True, stop=True,
                    tile_position=(64 * s_, 64 * s_))
            nc.scalar.activation(attnT[:, p_], at_ps[:], AF.Copy)
        if c == 0:
            dump("attnT0", attnT[:], [P, KT, CW], BF16)
            dump("qz0", qz[:], [P, KT, CW], BF16)

        # proj: out[t, e] = sum_d attnT[d, t] * wprojT[d, e]  (+ bias)
        for tt in range(TPC):
            t = c * TPC + tt
            o_ps = ops_.tile([P, D], F32)
            for kt in range(KT):
                for j, w_ in ((0, 512), (1, 256)):
                    mm = nc.tensor.matmul(
                        o_ps[:, ds(j * 512, w_)],
                        attnT[:, kt, ts(tt, P)],
                        wprojT[:, kt, ds(j * 512, w_)],
                        start=(kt == 0), stop=(kt == KT - 1))
                    if j > 0 and LDW_SKIP:
                        mm.ldweights = False  # same stationary as j-1
            osb = ph2.tile([P, D], F32, tag="osb")
            nc.vector.tensor_tensor(osb[:], o_ps[:], bias_sb[:], ALU.add)
            nc.sync.dma_start(out_d[ts(t, P), :], osb[:])


_CACHE = {}


def _get_nc(with_qkv_bias: bool, dbg: bool = False):
    key = ("nc", with_qkv_bias, dbg)
    if key not in _CACHE:
        _CACHE[key] = _build(with_qkv_bias, dbg)
    return _CACHE[key]


def kernel(x, ln_gamma, ln_beta, w_qkv, w_proj, b_proj, trace=False, dbg=False):
    x = np.asarray(x, dtype=np.float32)
    ln_gamma = np.asarray(ln_gamma, dtype=np.float32)
    ln_beta = np.asarray(ln_beta, dtype=np.float32)
    w_qkv = np.asarray(w_qkv, dtype=np.float32)
    w_proj = np.asarray(w_proj, dtype=np.float32)
    b_proj = np.asarray(b_proj, dtype=np.float32)
    bsz = x.shape[0]
    assert x.shape == (bsz, N, D) and bsz == N_CORES

    # Fold LN affine into the qkv projection (exact algebra):
    #   y = xhat*gamma + beta  =>  qkv = xhat @ (gamma*W)^T + W@beta
    wq_eff = (w_qkv * ln_gamma[None, :])          # [E3, D]
    cqkv = w_qkv @ ln_beta                        # [E3]
    with_bias = bool(np.any(cqkv))
    if with_bias:
        raise NotImplementedError(
            "nonzero W@beta path not wired into the device kernel")

    wqkvT = np.ascontiguousarray(wq_eff.T).astype(ml_dtypes.bfloat16)
    wprojT = np.ascontiguousarray(w_proj.T).astype(ml_dtypes.bfloat16)
    emat = np.zeros((H, D), dtype=ml_dtypes.bfloat16)
    for h in range(H):
        emat[h, h * HD:(h + 1) * HD] = 1
    bpack = np.concatenate([np.ones(P, np.float32),
                            b_proj.astype(np.float32)]).reshape(1, P + D)

    # If the caller's process pinned jax to cpu (common for reference
    # generation), re-discover the neuron/axon backend before the PJRT run.
    import jax
    if len(jax.devices()) < N_CORES:
        try:
            jax.config.update("jax_platforms", None)
            jax.clear_backends()
        except Exception:
            pass

    nc = _get_nc(with_bias, dbg)
    in_maps = []
    for i in range(N_CORES):
        m = {"x": np.ascontiguousarray(x[i]).astype(ml_dtypes.bfloat16),
             "wqkvT": wqkvT, "wprojT": wprojT, "emat": emat, "bpack": bpack}
        in_maps.append(m)

    res = run_bass_kernel_spmd(nc, in_maps, core_ids=list(range(N_CORES)),
                               trace=trace)
    out = np.stack([res.results[i]["out"] for i in range(N_CORES)], axis=0)
    if dbg:
        return out, res
    if trace:
        return out, res
    return out

